# revision 36
# baseline (speedup 1.0000x reference)
"""DMSAD loss kernel for Trainium2 (8 NeuronCores, data-parallel over batch).

Computes mean over B rows of:
    dist_i = max(min_j ||x_i - c_j||^2, 0)
    loss_i = dist_i                 if st_i == 0
             dist_i + EPS           if st_i == 1
             1 / (dist_i + EPS)     if st_i == -1

Strategy per core (B_SH = 16384 rows, D = 256, C = 128):
  - HWDGE DMA of x fp32 (1 MiB per 8-tile group) is the roofline driver
    (~51us/core).  Everything else hides under it.
  - DVE casts fp32 -> bf16 (2x_2P mode).
  - PE transposes 128x128 bf16 chunks via matmul-against-identity (keeps
    FWL + warm HAM clock); ACT/DVE copy-cast PSUM -> SBUF bf16.
  - DVE squares the transposed tiles (bf16 2x): xsqT.
  - Augmented matmul accumulates the FULL distance in PSUM:
        d2[i,j] = sum_d xsqT[d,i]*1  - 2 x.c^T  + c2_j
    via per tile: xT0.cT0, xT1.cT1, xsqT0.ones, xsqT1.ones, plus one
    K=2 N=512 c2 matmul per 4-tile PSUM group (bf16 hi/lo rows keep c2
    fp32-accurate).  No per-tile accumulator reads anywhere.
  - DVE min-reduces each PSUM group over centers -> dist columns.
  - Endgame on [128, NT]: relu, select by semi_target, row sums, one
    ones-matmul partition reduction to a scalar.
Host sums the 8 per-core partial sums and divides by global B.
"""

from contextlib import ExitStack, nullcontext

import numpy as np

import concourse.bass as bass
import concourse.tile as tile
from concourse import bacc, mybir
from concourse.bass_utils import run_bass_kernel_spmd
from concourse.masks import make_identity

N_CORES = 8
B = 131072
D = 256
C = 128
P = 128
B_SH = B // N_CORES          # 16384 rows per core
NT = B_SH // P               # 128 b-tiles of 128 rows
PSUM_GROUP = 4               # b-tiles per PSUM batch (one G bank)
DMA_GROUP = 8                # b-tiles per input DMA (1 MiB fp32 reads)
NG = NT // PSUM_GROUP        # 32 psum groups
ETA = 1.0
EPS = 1e-6

# engine balancing knobs -------------------------------------------------
# PSUM->SBUF copy-cast of transposed x: out of 10 groups, this many on ACT
# (rest on DVE).
COPY_ACT_OF_10 = 10
# xsq squaring: out of 10 groups, this many on DVE (rest on ACT Square).
SQ_DVE_OF_10 = 6
# fp32->bf16 input cast: out of 10 DMA-groups, this many on DVE (rest ACT)
CAST_DVE_OF_10 = 10
# fold xsq chunk pairs (xsq0+xsq1) so one N=128 matmul injects x2: engine
# "gpsimd" (idle engine), "dve", or "" to keep two x2-matmuls per tile
X2_FOLD = ""
# ACT-side squares read the transposed tiles from PSUM (runs parallel to
# the PSUM->SBUF copy instead of after it)
SQ_FROM_PSUM = True
# number of x DMA groups issued before the constants prep
PREFETCH = 4
# software-pipeline skew (groups) between transpose-MMs and G-MMs on PE
SKEW = 3
# the last XBAR_LAST psum-groups transpose via the DMA xbar (the SDMA
# engines are idle once the x loads finish) instead of PE matmuls + copy
XBAR_LAST = 6

F32 = mybir.dt.float32
BF16 = mybir.dt.bfloat16
AF = mybir.ActivationFunctionType
ALU = mybir.AluOpType

_cached_nc = {}


def _emit(ctx: ExitStack, tc, x_d, c_d, st_d, out_d, repeat: int = 1,
          hw_loop: int = 1):
    nc = tc.nc

    const = ctx.enter_context(tc.tile_pool(name="const", bufs=1))
    xpool = ctx.enter_context(tc.tile_pool(name="xin", bufs=PREFETCH + 2))
    xtps = ctx.enter_context(tc.tile_pool(name="xtps", bufs=2, space="PSUM"))
    xtsb = ctx.enter_context(tc.tile_pool(name="xtsb", bufs=SKEW + 3))
    xsqp = ctx.enter_context(tc.tile_pool(name="xsq", bufs=SKEW + 3))
    gps = ctx.enter_context(tc.tile_pool(name="gps", bufs=3, space="PSUM"))
    scr_ps = ctx.enter_context(tc.tile_pool(name="scrps", bufs=1, space="PSUM"))
    endp = ctx.enter_context(tc.tile_pool(name="endp", bufs=1))

    # ---- prefetch first x groups before anything else ------------------
    def x_src(gd):
        src = x_d[gd * DMA_GROUP * P:(gd + 1) * DMA_GROUP * P, :]
        # row (p, t) of group gd = batch gd*1024 + p*8 + t: each partition
        # reads one contiguous 8 KiB run per DMA
        return src.rearrange("(p t) d -> p t d", t=DMA_GROUP)

    prefetched = {}
    for gd in range(PREFETCH):
        xf8 = xpool.tile([P, DMA_GROUP, D], F32, tag="xf")
        nc.sync.dma_start(xf8[:], x_src(gd))
        prefetched[gd] = xf8

    # c + st load on the scalar HWDGE ring: independent FIFO, so they are
    # not stuck behind the MiB-scale x prefetches on the sync ring
    c_sb = const.tile([C, D], F32)
    nc.scalar.dma_start(c_sb[:], c_d[:])

    # ---- one-time prep -------------------------------------------------
    ident_bf = const.tile([P, P], BF16)
    make_identity(nc, ident_bf[:])

    # c2 = rowsum(c^2) as a [128, 1] fp32 column
    c_sq = const.tile([C, D], F32)
    c2col = const.tile([C, 1], F32)
    nc.scalar.activation(c_sq[:], c_sb[:], AF.Square, accum_out=c2col[:])

    # (-2c) in bf16, then its transpose cT [d-chunk partitions, k, centers]
    cm2 = const.tile([C, D], BF16)
    nc.vector.tensor_scalar_mul(cm2[:], c_sb[:], -2.0)
    ct_ps = scr_ps.tile([P, 2, C], BF16, tag="scratch")
    for k in range(2):
        nc.tensor.transpose(ct_ps[:, k, :], cm2[:, k * P:(k + 1) * P], ident_bf[:])
    cT = const.tile([P, 2, C], BF16)
    nc.vector.tensor_copy(cT[:], ct_ps[:])

    # c2 as two bf16 K-rows (hi + lo) so a K=2 ones-matmul adds fp32-accurate
    # c2.  Build hi/lo as COLUMNS of a [C, 2] tile (engines can write any
    # free offset but not partition base 1), then one PE transpose makes the
    # [2, C] row pair.  No DMA: a tiny SBUF->SBUF DMA here gets stuck for
    # ~15us behind the MiB-scale x-load packets on the shared SDMA engines.
    c2cols = const.tile([C, 2], BF16)
    nc.vector.tensor_copy(c2cols[:, 0:1], c2col[:])
    c2hi_f = const.tile([C, 1], F32)
    nc.vector.tensor_copy(c2hi_f[:], c2cols[:, 0:1])
    c2lo_f = const.tile([C, 1], F32)
    nc.vector.tensor_tensor(c2lo_f[:], c2col[:], c2hi_f[:], op=ALU.subtract)
    nc.vector.tensor_copy(c2cols[:, 1:2], c2lo_f[:])
    c2t_ps = scr_ps.tile([2, C], F32, tag="scratch")
    nc.tensor.matmul(c2t_ps[:], lhsT=c2cols[:], rhs=ident_bf[:])
    c2rows = const.tile([2, C], BF16)
    nc.vector.tensor_copy(c2rows[:], c2t_ps[:])

    ones2 = const.tile([2, C], BF16)
    nc.vector.memset(ones2[:], 1.0)
    ones_col = const.tile([P, 1], F32)
    nc.vector.memset(ones_col[:], 1.0)
    # all-ones [d, j] rhs for the xsqT matmuls that inject x2 into PSUM
    ones_t = const.tile([P, C], BF16)
    nc.vector.memset(ones_t[:], 1.0)

    # c2rows replicated PSUM_GROUP times for the single N=512 c2 matmul
    c2rows4 = const.tile([2, PSUM_GROUP, C], BF16)
    for i in range(PSUM_GROUP):
        nc.vector.tensor_copy(c2rows4[:, i, :], c2rows[:])

    # semi_target: host pre-reorders it to the x row mapping (batch row
    # i = g*(DMA_GROUP*P) + p*DMA_GROUP + t at st_sb[p, g*DMA_GROUP + t]),
    # so the device load is 512 B contiguous per partition.  The naive
    # gather layout is 32 B/descriptor and crawls behind the x packets.
    st_sb = const.tile([P, NT], F32)
    nc.scalar.dma_start(st_sb[:], st_d.rearrange("(p n) -> p n", p=P))
    # st-derived endgame operands, precomputed while the pipeline is cold
    mneg = const.tile([P, NT], F32)
    nc.vector.tensor_scalar_min(mneg[:], st_sb[:], 0.0)
    epsq = const.tile([P, NT], F32)
    nc.vector.tensor_scalar(epsq[:], st_sb[:], 0.0, EPS, op0=ALU.max, op1=ALU.mult)

    # per-b-tile min columns: column j <-> b-tile j, partition p <-> row
    dw = const.tile([P, NT], F32)

    # ---- main loop -----------------------------------------------------
    pending = []  # (group_idx, xt_sb, xsq) awaiting G-matmuls
    gdone = []    # (group_idx, g_ps) awaiting min-reduce

    def emit_g(g, xt_sb, xsq):
        folded = xsq.shape[2] == 1
        g_ps = gps.tile([P, PSUM_GROUP, C], F32)
        nc.tensor.matmul(
            g_ps[:].rearrange("p t c -> p (t c)"),
            lhsT=ones2[:], rhs=c2rows4[:].rearrange("p t c -> p (t c)"),
            start=True, stop=False,
        )
        for i in range(PSUM_GROUP):
            nc.tensor.matmul(
                g_ps[:, i, :], lhsT=xt_sb[:, i, 0, :], rhs=cT[:, 0, :],
                start=False, stop=False,
            )
            nc.tensor.matmul(
                g_ps[:, i, :], lhsT=xt_sb[:, i, 1, :], rhs=cT[:, 1, :],
                start=False, stop=False,
            )
            nc.tensor.matmul(
                g_ps[:, i, :], lhsT=xsq[:, i, 0, :], rhs=ones_t[:],
                start=False, stop=(folded and i == PSUM_GROUP - 1),
            )
            if not folded:
                nc.tensor.matmul(
                    g_ps[:, i, :], lhsT=xsq[:, i, 1, :], rhs=ones_t[:],
                    start=False, stop=(i == PSUM_GROUP - 1),
                )
        gdone.append((g, g_ps))

    def emit_min(g, g_ps):
        col0 = g * PSUM_GROUP
        nc.vector.tensor_reduce(
            dw[:, col0:col0 + PSUM_GROUP], g_ps[:], axis=mybir.AxisListType.X,
            op=ALU.min,
        )

    with tc.For_i(0, hw_loop, 1) if hw_loop > 1 else nullcontext():
     for _rep in range(repeat):
      for gd in range(NT // DMA_GROUP):
        if gd in prefetched:
            xf8 = prefetched.pop(gd)
        else:
            xf8 = xpool.tile([P, DMA_GROUP, D], F32, tag="xf")
            nc.sync.dma_start(xf8[:], x_src(gd))
        x8 = xpool.tile([P, DMA_GROUP, D], BF16, tag="xb")
        if (gd % 10) < CAST_DVE_OF_10:
            nc.vector.tensor_copy(x8[:], xf8[:])
        else:
            nc.scalar.copy(x8[:], xf8[:])

        for gp in range(DMA_GROUP // PSUM_GROUP):
            g = gd * (DMA_GROUP // PSUM_GROUP) + gp
            tiles = [gp * PSUM_GROUP + t for t in range(PSUM_GROUP)]

            use_xbar = g >= 2 * (NT // DMA_GROUP) - XBAR_LAST
            xt_sb = xtsb.tile([P, PSUM_GROUP, 2, P], BF16)
            if use_xbar:
                xt_ps = None
                nc.sync.dma_start_transpose(
                    xt_sb[:].rearrange("p t k b -> p (t k) b"),
                    x8[:, gp * PSUM_GROUP:(gp + 1) * PSUM_GROUP, :]
                    .rearrange("p t d -> p (t d)"),
                )
            else:
                xt_ps = xtps.tile([P, PSUM_GROUP, 2, P], F32)
                for i, t in enumerate(tiles):
                    for k in range(2):
                        nc.tensor.matmul(
                            xt_ps[:, i, k, :],
                            lhsT=x8[:, t, k * P:(k + 1) * P],
                            rhs=ident_bf[:],
                        )
                if (g % 10) < COPY_ACT_OF_10:
                    nc.scalar.copy(xt_sb[:], xt_ps[:])
                else:
                    nc.vector.tensor_copy(xt_sb[:], xt_ps[:])

            xsq = xsqp.tile([P, PSUM_GROUP, 2, P], BF16)
            if (g % 10) < SQ_DVE_OF_10:
                nc.vector.tensor_tensor(xsq[:], xt_sb[:], xt_sb[:], op=ALU.mult)
            elif SQ_FROM_PSUM and xt_ps is not None:
                nc.scalar.activation(xsq[:], xt_ps[:], AF.Square)
            else:
                nc.scalar.activation(xsq[:], xt_sb[:], AF.Square)
            if X2_FOLD:
                xsq_f = xsqp.tile([P, PSUM_GROUP, 1, P], BF16, tag="xsqf")
                eng = nc.gpsimd if X2_FOLD == "gpsimd" else nc.vector
                eng.tensor_tensor(
                    xsq_f[:, :, 0, :], xsq[:, :, 0, :], xsq[:, :, 1, :],
                    op=ALU.add,
                )
                xsq = xsq_f

            pending.append((g, xt_sb, xsq))
            if len(pending) > SKEW:
                emit_g(*pending.pop(0))
            while len(gdone) > 1:
                emit_min(*gdone.pop(0))

      while pending:
        emit_g(*pending.pop(0))
      while gdone:
        emit_min(*gdone.pop(0))

    # ---- endgame -------------------------------------------------------
    dist = endp.tile([P, NT], F32)
    nc.vector.tensor_scalar_max(dist[:], dw[:], 0.0)
    dT = dist

    dp = endp.tile([P, NT], F32)
    nc.vector.tensor_scalar_add(dp[:], dT[:], EPS)
    r = endp.tile([P, NT], F32)
    nc.vector.reciprocal(r[:], dp[:])

    # loss = dT + min(st,0)*(dT - r) + max(st,0)*EPS
    t1 = endp.tile([P, NT], F32)
    nc.vector.tensor_tensor(t1[:], dT[:], r[:], op=ALU.subtract)
    t2 = endp.tile([P, NT], F32)
    nc.vector.tensor_tensor(t2[:], mneg[:], t1[:], op=ALU.mult)
    t3 = endp.tile([P, NT], F32)
    nc.vector.tensor_tensor(t3[:], dT[:], t2[:], op=ALU.add)
    losses = endp.tile([P, NT], F32)
    nc.vector.tensor_tensor(losses[:], t3[:], epsq[:], op=ALU.add)

    lsum = endp.tile([P, 1], F32)
    nc.vector.tensor_reduce(lsum[:], losses[:], axis=mybir.AxisListType.X, op=ALU.add)
    total_ps = scr_ps.tile([1, 1], F32, tag="scratch")
    nc.tensor.matmul(total_ps[:], lhsT=ones_col[:], rhs=lsum[:])
    total_sb = endp.tile([1, 1], F32)
    nc.vector.tensor_copy(total_sb[:], total_ps[:])
    nc.sync.dma_start(out_d[:], total_sb[:])


def build_nc(repeat: int = 1, hw_loop: int = 1, internal_x: bool = False):
    key = (repeat, hw_loop, internal_x)
    if key in _cached_nc:
        return _cached_nc[key]
    nc = bacc.Bacc(
        "TRN2",
        target_bir_lowering=False,
        debug=False,
        enable_asserts=False,
        num_devices=N_CORES,
    )
    if internal_x:
        x_d = nc.dram_tensor("x", [B_SH, D], F32).ap()
    else:
        x_d = nc.dram_tensor("x", [B_SH, D], F32, kind="ExternalInput").ap()
    c_d = nc.dram_tensor("c", [C, D], F32, kind="ExternalInput").ap()
    st_d = nc.dram_tensor("st", [B_SH], F32, kind="ExternalInput").ap()
    out_d = nc.dram_tensor("out", [1, 1], F32, kind="ExternalOutput").ap()

    with tile.TileContext(nc) as tc:
        with ExitStack() as ctx:
            _emit(ctx, tc, x_d, c_d, st_d, out_d, repeat=repeat, hw_loop=hw_loop)
    nc.compile()
    _cached_nc[key] = nc
    return nc


def make_in_maps(x, c, stf):
    def st_reorder(s):
        # st_sb[p, g*DMA_GROUP + t] = s[g*(DMA_GROUP*P) + p*DMA_GROUP + t]
        return np.ascontiguousarray(
            s.reshape(NT // DMA_GROUP, P, DMA_GROUP)
            .transpose(1, 0, 2)
            .reshape(B_SH)
        )

    return [
        {
            "x": np.ascontiguousarray(x[i * B_SH:(i + 1) * B_SH]),
            "c": c,
            "st": st_reorder(stf[i * B_SH:(i + 1) * B_SH]),
        }
        for i in range(N_CORES)
    ]


def kernel(**inputs) -> np.ndarray:
    x = np.ascontiguousarray(np.asarray(inputs["input"], dtype=np.float32))
    c = np.ascontiguousarray(np.asarray(inputs["c"], dtype=np.float32))
    stf = np.asarray(inputs["semi_target"]).astype(np.float32)

    nc = build_nc()
    res = run_bass_kernel_spmd(nc, make_in_maps(x, c, stf), list(range(N_CORES)))
    total = sum(float(r["out"][0, 0]) for r in res.results)
    return np.asarray(np.float32(total / B))


# revision 38
# speedup vs baseline: 1.0126x; 1.0126x over previous
"""DMSAD loss kernel for Trainium2 (8 NeuronCores, data-parallel over batch).

Computes mean over B rows of:
    dist_i = max(min_j ||x_i - c_j||^2, 0)
    loss_i = dist_i                 if st_i == 0
             dist_i + EPS           if st_i == 1
             1 / (dist_i + EPS)     if st_i == -1

Strategy per core (B_SH = 16384 rows, D = 256, C = 128):
  - HWDGE DMA of x fp32 (1 MiB per 8-tile group) is the roofline driver
    (~51us/core).  Everything else hides under it.
  - DVE casts fp32 -> bf16 (2x_2P mode).
  - PE transposes 128x128 bf16 chunks via matmul-against-identity (keeps
    FWL + warm HAM clock); ACT/DVE copy-cast PSUM -> SBUF bf16.
  - DVE squares the transposed tiles (bf16 2x): xsqT.
  - Augmented matmul accumulates the FULL distance in PSUM:
        d2[i,j] = sum_d xsqT[d,i]*1  - 2 x.c^T  + c2_j
    via per tile: xT0.cT0, xT1.cT1, xsqT0.ones, xsqT1.ones, plus one
    K=2 N=512 c2 matmul per 4-tile PSUM group (bf16 hi/lo rows keep c2
    fp32-accurate).  No per-tile accumulator reads anywhere.
  - DVE min-reduces each PSUM group over centers -> dist columns.
  - Endgame on [128, NT]: relu, select by semi_target, row sums, one
    ones-matmul partition reduction to a scalar.
Host sums the 8 per-core partial sums and divides by global B.
"""

from contextlib import ExitStack, nullcontext

import numpy as np

import concourse.bass as bass
import concourse.tile as tile
from concourse import bacc, mybir
from concourse.bass_utils import run_bass_kernel_spmd
from concourse.masks import make_identity

N_CORES = 8
B = 131072
D = 256
C = 128
P = 128
B_SH = B // N_CORES          # 16384 rows per core
NT = B_SH // P               # 128 b-tiles of 128 rows
PSUM_GROUP = 4               # b-tiles per PSUM batch (one G bank)
DMA_GROUP = 8                # b-tiles per input DMA (1 MiB fp32 reads)
NG = NT // PSUM_GROUP        # 32 psum groups
ETA = 1.0
EPS = 1e-6

# engine balancing knobs -------------------------------------------------
# PSUM->SBUF copy-cast of transposed x: out of 10 groups, this many on ACT
# (rest on DVE).
COPY_ACT_OF_10 = 10
# xsq squaring: out of 10 groups, this many on DVE (rest on ACT Square).
SQ_DVE_OF_10 = 3
# fp32->bf16 input cast: out of 10 DMA-groups, this many on DVE (rest ACT)
CAST_DVE_OF_10 = 10
# fold xsq chunk pairs (xsq0+xsq1) so one N=128 matmul injects x2: engine
# "gpsimd" (idle engine), "dve", or "" to keep two x2-matmuls per tile
X2_FOLD = ""
# ACT-side squares read the transposed tiles from PSUM (runs parallel to
# the PSUM->SBUF copy instead of after it)
SQ_FROM_PSUM = True
# number of x DMA groups issued before the constants prep
PREFETCH = 4
# software-pipeline skew (groups) between transpose-MMs and G-MMs on PE
SKEW = 3
# the last XBAR_LAST psum-groups transpose via the DMA xbar (the SDMA
# engines are idle once the x loads finish) instead of PE matmuls + copy
# (measured: a net loss on this workload — keep 0)
XBAR_LAST = 0

F32 = mybir.dt.float32
BF16 = mybir.dt.bfloat16
AF = mybir.ActivationFunctionType
ALU = mybir.AluOpType

_cached_nc = {}


def _emit(ctx: ExitStack, tc, x_d, c_d, st_d, out_d, repeat: int = 1,
          hw_loop: int = 1):
    nc = tc.nc

    const = ctx.enter_context(tc.tile_pool(name="const", bufs=1))
    xpool = ctx.enter_context(tc.tile_pool(name="xin", bufs=PREFETCH + 2))
    xtps = ctx.enter_context(tc.tile_pool(name="xtps", bufs=2, space="PSUM"))
    xtsb = ctx.enter_context(tc.tile_pool(name="xtsb", bufs=SKEW + 3))
    xsqp = ctx.enter_context(tc.tile_pool(name="xsq", bufs=SKEW + 3))
    gps = ctx.enter_context(tc.tile_pool(name="gps", bufs=3, space="PSUM"))
    scr_ps = ctx.enter_context(tc.tile_pool(name="scrps", bufs=1, space="PSUM"))
    endp = ctx.enter_context(tc.tile_pool(name="endp", bufs=1))

    # ---- prefetch first x groups before anything else ------------------
    def x_src(gd):
        src = x_d[gd * DMA_GROUP * P:(gd + 1) * DMA_GROUP * P, :]
        # row (p, t) of group gd = batch gd*1024 + p*8 + t: each partition
        # reads one contiguous 8 KiB run per DMA
        return src.rearrange("(p t) d -> p t d", t=DMA_GROUP)

    prefetched = {}
    for gd in range(PREFETCH):
        xf8 = xpool.tile([P, DMA_GROUP, D], F32, tag="xf")
        nc.sync.dma_start(xf8[:], x_src(gd))
        prefetched[gd] = xf8

    # c + st load on the scalar HWDGE ring: independent FIFO, so they are
    # not stuck behind the MiB-scale x prefetches on the sync ring
    c_sb = const.tile([C, D], F32)
    nc.scalar.dma_start(c_sb[:], c_d[:])

    # ---- one-time prep -------------------------------------------------
    ident_bf = const.tile([P, P], BF16)
    make_identity(nc, ident_bf[:])

    # c2 = rowsum(c^2) as a [128, 1] fp32 column
    c_sq = const.tile([C, D], F32)
    c2col = const.tile([C, 1], F32)
    nc.scalar.activation(c_sq[:], c_sb[:], AF.Square, accum_out=c2col[:])

    # (-2c) in bf16, then its transpose cT [d-chunk partitions, k, centers]
    cm2 = const.tile([C, D], BF16)
    nc.vector.tensor_scalar_mul(cm2[:], c_sb[:], -2.0)
    ct_ps = scr_ps.tile([P, 2, C], BF16, tag="scratch")
    for k in range(2):
        nc.tensor.transpose(ct_ps[:, k, :], cm2[:, k * P:(k + 1) * P], ident_bf[:])
    cT = const.tile([P, 2, C], BF16)
    nc.vector.tensor_copy(cT[:], ct_ps[:])

    # c2 as two bf16 K-rows (hi + lo) so a K=2 ones-matmul adds fp32-accurate
    # c2.  Build hi/lo as COLUMNS of a [C, 2] tile (engines can write any
    # free offset but not partition base 1), then one PE transpose makes the
    # [2, C] row pair.  No DMA: a tiny SBUF->SBUF DMA here gets stuck for
    # ~15us behind the MiB-scale x-load packets on the shared SDMA engines.
    c2cols = const.tile([C, 2], BF16)
    nc.vector.tensor_copy(c2cols[:, 0:1], c2col[:])
    c2hi_f = const.tile([C, 1], F32)
    nc.vector.tensor_copy(c2hi_f[:], c2cols[:, 0:1])
    c2lo_f = const.tile([C, 1], F32)
    nc.vector.tensor_tensor(c2lo_f[:], c2col[:], c2hi_f[:], op=ALU.subtract)
    nc.vector.tensor_copy(c2cols[:, 1:2], c2lo_f[:])
    c2t_ps = scr_ps.tile([2, C], F32, tag="scratch")
    nc.tensor.matmul(c2t_ps[:], lhsT=c2cols[:], rhs=ident_bf[:])
    c2rows = const.tile([2, C], BF16)
    nc.vector.tensor_copy(c2rows[:], c2t_ps[:])

    ones2 = const.tile([2, C], BF16)
    nc.vector.memset(ones2[:], 1.0)
    ones_col = const.tile([P, 1], F32)
    nc.vector.memset(ones_col[:], 1.0)
    # all-ones [d, j] rhs for the xsqT matmuls that inject x2 into PSUM
    ones_t = const.tile([P, C], BF16)
    nc.vector.memset(ones_t[:], 1.0)

    # c2rows replicated PSUM_GROUP times for the single N=512 c2 matmul
    c2rows4 = const.tile([2, PSUM_GROUP, C], BF16)
    for i in range(PSUM_GROUP):
        nc.vector.tensor_copy(c2rows4[:, i, :], c2rows[:])

    # semi_target: host pre-reorders it to the x row mapping (batch row
    # i = g*(DMA_GROUP*P) + p*DMA_GROUP + t at st_sb[p, g*DMA_GROUP + t]),
    # so the device load is 512 B contiguous per partition.  The naive
    # gather layout is 32 B/descriptor and crawls behind the x packets.
    st_sb = const.tile([P, NT], F32)
    nc.scalar.dma_start(st_sb[:], st_d.rearrange("(p n) -> p n", p=P))
    # st-derived endgame operands, precomputed while the pipeline is cold
    mneg = const.tile([P, NT], F32)
    nc.vector.tensor_scalar_min(mneg[:], st_sb[:], 0.0)
    epsq = const.tile([P, NT], F32)
    nc.vector.tensor_scalar(epsq[:], st_sb[:], 0.0, EPS, op0=ALU.max, op1=ALU.mult)

    # per-b-tile min columns: column j <-> b-tile j, partition p <-> row
    dw = const.tile([P, NT], F32)

    # ---- main loop -----------------------------------------------------
    pending = []  # (group_idx, xt_sb, xsq) awaiting G-matmuls
    gdone = []    # (group_idx, g_ps) awaiting min-reduce

    def emit_g(g, xt_sb, xsq):
        folded = xsq.shape[2] == 1
        g_ps = gps.tile([P, PSUM_GROUP, C], F32)
        nc.tensor.matmul(
            g_ps[:].rearrange("p t c -> p (t c)"),
            lhsT=ones2[:], rhs=c2rows4[:].rearrange("p t c -> p (t c)"),
            start=True, stop=False,
        )
        for i in range(PSUM_GROUP):
            nc.tensor.matmul(
                g_ps[:, i, :], lhsT=xt_sb[:, i, 0, :], rhs=cT[:, 0, :],
                start=False, stop=False,
            )
            nc.tensor.matmul(
                g_ps[:, i, :], lhsT=xt_sb[:, i, 1, :], rhs=cT[:, 1, :],
                start=False, stop=False,
            )
            nc.tensor.matmul(
                g_ps[:, i, :], lhsT=xsq[:, i, 0, :], rhs=ones_t[:],
                start=False, stop=(folded and i == PSUM_GROUP - 1),
            )
            if not folded:
                nc.tensor.matmul(
                    g_ps[:, i, :], lhsT=xsq[:, i, 1, :], rhs=ones_t[:],
                    start=False, stop=(i == PSUM_GROUP - 1),
                )
        gdone.append((g, g_ps))

    def emit_min(g, g_ps):
        col0 = g * PSUM_GROUP
        nc.vector.tensor_reduce(
            dw[:, col0:col0 + PSUM_GROUP], g_ps[:], axis=mybir.AxisListType.X,
            op=ALU.min,
        )

    with tc.For_i(0, hw_loop, 1) if hw_loop > 1 else nullcontext():
     for _rep in range(repeat):
      for gd in range(NT // DMA_GROUP):
        if gd in prefetched:
            xf8 = prefetched.pop(gd)
        else:
            xf8 = xpool.tile([P, DMA_GROUP, D], F32, tag="xf")
            nc.sync.dma_start(xf8[:], x_src(gd))
        x8 = xpool.tile([P, DMA_GROUP, D], BF16, tag="xb")
        if (gd % 10) < CAST_DVE_OF_10:
            nc.vector.tensor_copy(x8[:], xf8[:])
        else:
            nc.scalar.copy(x8[:], xf8[:])

        for gp in range(DMA_GROUP // PSUM_GROUP):
            g = gd * (DMA_GROUP // PSUM_GROUP) + gp
            tiles = [gp * PSUM_GROUP + t for t in range(PSUM_GROUP)]

            use_xbar = g >= 2 * (NT // DMA_GROUP) - XBAR_LAST
            xt_sb = xtsb.tile([P, PSUM_GROUP, 2, P], BF16)
            if use_xbar:
                xt_ps = None
                nc.sync.dma_start_transpose(
                    xt_sb[:].rearrange("p t k b -> p (t k) b"),
                    x8[:, gp * PSUM_GROUP:(gp + 1) * PSUM_GROUP, :]
                    .rearrange("p t d -> p (t d)"),
                )
            else:
                xt_ps = xtps.tile([P, PSUM_GROUP, 2, P], F32)
                for i, t in enumerate(tiles):
                    for k in range(2):
                        nc.tensor.matmul(
                            xt_ps[:, i, k, :],
                            lhsT=x8[:, t, k * P:(k + 1) * P],
                            rhs=ident_bf[:],
                        )
                if (g % 10) < COPY_ACT_OF_10:
                    nc.scalar.copy(xt_sb[:], xt_ps[:])
                else:
                    nc.vector.tensor_copy(xt_sb[:], xt_ps[:])

            xsq = xsqp.tile([P, PSUM_GROUP, 2, P], BF16)
            if (g % 10) < SQ_DVE_OF_10:
                nc.vector.tensor_tensor(xsq[:], xt_sb[:], xt_sb[:], op=ALU.mult)
            elif SQ_FROM_PSUM and xt_ps is not None:
                nc.scalar.activation(xsq[:], xt_ps[:], AF.Square)
            else:
                nc.scalar.activation(xsq[:], xt_sb[:], AF.Square)
            if X2_FOLD:
                xsq_f = xsqp.tile([P, PSUM_GROUP, 1, P], BF16, tag="xsqf")
                eng = nc.gpsimd if X2_FOLD == "gpsimd" else nc.vector
                eng.tensor_tensor(
                    xsq_f[:, :, 0, :], xsq[:, :, 0, :], xsq[:, :, 1, :],
                    op=ALU.add,
                )
                xsq = xsq_f

            pending.append((g, xt_sb, xsq))
            if len(pending) > SKEW:
                emit_g(*pending.pop(0))
            while len(gdone) > 1:
                emit_min(*gdone.pop(0))

      while pending:
        emit_g(*pending.pop(0))
      while gdone:
        emit_min(*gdone.pop(0))

    # ---- endgame -------------------------------------------------------
    dist = endp.tile([P, NT], F32)
    nc.vector.tensor_scalar_max(dist[:], dw[:], 0.0)
    dT = dist

    dp = endp.tile([P, NT], F32)
    nc.vector.tensor_scalar_add(dp[:], dT[:], EPS)
    r = endp.tile([P, NT], F32)
    nc.vector.reciprocal(r[:], dp[:])

    # loss = dT + min(st,0)*(dT - r) + max(st,0)*EPS
    t1 = endp.tile([P, NT], F32)
    nc.vector.tensor_tensor(t1[:], dT[:], r[:], op=ALU.subtract)
    t2 = endp.tile([P, NT], F32)
    nc.vector.tensor_tensor(t2[:], mneg[:], t1[:], op=ALU.mult)
    t3 = endp.tile([P, NT], F32)
    nc.vector.tensor_tensor(t3[:], dT[:], t2[:], op=ALU.add)
    losses = endp.tile([P, NT], F32)
    nc.vector.tensor_tensor(losses[:], t3[:], epsq[:], op=ALU.add)

    lsum = endp.tile([P, 1], F32)
    nc.vector.tensor_reduce(lsum[:], losses[:], axis=mybir.AxisListType.X, op=ALU.add)
    total_ps = scr_ps.tile([1, 1], F32, tag="scratch")
    nc.tensor.matmul(total_ps[:], lhsT=ones_col[:], rhs=lsum[:])
    total_sb = endp.tile([1, 1], F32)
    nc.vector.tensor_copy(total_sb[:], total_ps[:])
    nc.sync.dma_start(out_d[:], total_sb[:])


def build_nc(repeat: int = 1, hw_loop: int = 1, internal_x: bool = False):
    key = (repeat, hw_loop, internal_x)
    if key in _cached_nc:
        return _cached_nc[key]
    nc = bacc.Bacc(
        "TRN2",
        target_bir_lowering=False,
        debug=False,
        enable_asserts=False,
        num_devices=N_CORES,
    )
    if internal_x:
        x_d = nc.dram_tensor("x", [B_SH, D], F32).ap()
    else:
        x_d = nc.dram_tensor("x", [B_SH, D], F32, kind="ExternalInput").ap()
    c_d = nc.dram_tensor("c", [C, D], F32, kind="ExternalInput").ap()
    st_d = nc.dram_tensor("st", [B_SH], F32, kind="ExternalInput").ap()
    out_d = nc.dram_tensor("out", [1, 1], F32, kind="ExternalOutput").ap()

    with tile.TileContext(nc) as tc:
        with ExitStack() as ctx:
            _emit(ctx, tc, x_d, c_d, st_d, out_d, repeat=repeat, hw_loop=hw_loop)
    nc.compile()
    _cached_nc[key] = nc
    return nc


def make_in_maps(x, c, stf):
    def st_reorder(s):
        # st_sb[p, g*DMA_GROUP + t] = s[g*(DMA_GROUP*P) + p*DMA_GROUP + t]
        return np.ascontiguousarray(
            s.reshape(NT // DMA_GROUP, P, DMA_GROUP)
            .transpose(1, 0, 2)
            .reshape(B_SH)
        )

    return [
        {
            "x": np.ascontiguousarray(x[i * B_SH:(i + 1) * B_SH]),
            "c": c,
            "st": st_reorder(stf[i * B_SH:(i + 1) * B_SH]),
        }
        for i in range(N_CORES)
    ]


def kernel(**inputs) -> np.ndarray:
    x = np.ascontiguousarray(np.asarray(inputs["input"], dtype=np.float32))
    c = np.ascontiguousarray(np.asarray(inputs["c"], dtype=np.float32))
    stf = np.asarray(inputs["semi_target"]).astype(np.float32)

    nc = build_nc()
    res = run_bass_kernel_spmd(nc, make_in_maps(x, c, stf), list(range(N_CORES)))
    total = sum(float(r["out"][0, 0]) for r in res.results)
    return np.asarray(np.float32(total / B))


# revision 42
# speedup vs baseline: 1.0311x; 1.0183x over previous
"""DMSAD loss kernel for Trainium2 (8 NeuronCores, data-parallel over batch).

Computes mean over B rows of:
    dist_i = max(min_j ||x_i - c_j||^2, 0)
    loss_i = dist_i                 if st_i == 0
             dist_i + EPS           if st_i == 1
             1 / (dist_i + EPS)     if st_i == -1

Strategy per core (B_SH = 16384 rows, D = 256, C = 128):
  - HWDGE DMA of x fp32 (1 MiB per 8-tile group, 4 groups prefetched,
    first group split for a fast start) is the roofline driver
    (~47-51us/core).  c and semi_target load on the scalar HWDGE ring so
    they are not queued behind the x prefetches; semi_target is
    pre-reordered on the host so its load is 512 B/partition contiguous
    (the gather layout was 32 B/descriptor and crawled behind x packets).
  - DVE casts fp32 -> bf16 (2x_2P mode).
  - PE transposes 128x128 bf16 chunks via matmul-against-identity (keeps
    FWL + warm HAM clock; ~56 ns/pair warm); ACT copy-casts PSUM -> SBUF
    bf16.
  - Squares of the transposed tiles (bf16): DVE tensor_tensor from SBUF
    for 6/10 groups, ACT Square straight from PSUM (parallel with the
    copy) for the rest.
  - Augmented matmul accumulates the FULL distance in PSUM:
        d2[i,j] = sum_d xsqT[d,i]*1  - 2 x.c^T  + c2_j
    via per tile: xT0.cT0, xT1.cT1, xsqT0.ones, xsqT1.ones, plus one
    K=2 N=512 c2 matmul per 4-tile PSUM group (bf16 hi/lo rows keep c2
    fp32-accurate; built via a PE transpose of hi/lo columns -- no DMA).
    No per-tile accumulator reads anywhere.  G-matmuls trail the
    transposes by SKEW groups so the PE stream never stalls on the
    copy/square latency.
  - DVE min-reduces each PSUM group over centers -> dist columns.
  - Endgame on [128, NT]: relu, select by semi_target (st-derived masks
    precomputed at startup), row sums, one ones-matmul partition
    reduction to a scalar.
Host sums the 8 per-core partial sums and divides by global B.
"""

from contextlib import ExitStack, nullcontext

import numpy as np

import concourse.bass as bass
import concourse.tile as tile
from concourse import bacc, mybir
from concourse.bass_utils import run_bass_kernel_spmd
from concourse.masks import make_identity

N_CORES = 8
B = 131072
D = 256
C = 128
P = 128
B_SH = B // N_CORES          # 16384 rows per core
NT = B_SH // P               # 128 b-tiles of 128 rows
PSUM_GROUP = 4               # b-tiles per PSUM batch (one G bank)
DMA_GROUP = 8                # b-tiles per input DMA (1 MiB fp32 reads)
NG = NT // PSUM_GROUP        # 32 psum groups
ETA = 1.0
EPS = 1e-6

# engine balancing knobs -------------------------------------------------
# PSUM->SBUF copy-cast of transposed x: out of 10 groups, this many on ACT
# (rest on DVE).
COPY_ACT_OF_10 = 10
# xsq squaring: out of 10 groups, this many on DVE (rest on ACT Square).
SQ_DVE_OF_10 = 6
# fp32->bf16 input cast: out of 10 DMA-groups, this many on DVE (rest ACT)
CAST_DVE_OF_10 = 10
# fold xsq chunk pairs (xsq0+xsq1) so one N=128 matmul injects x2: engine
# "gpsimd" (idle engine), "dve", or "" to keep two x2-matmuls per tile
X2_FOLD = ""
# ACT-side squares read the transposed tiles from PSUM (runs parallel to
# the PSUM->SBUF copy instead of after it)
SQ_FROM_PSUM = True
# number of x DMA groups issued before the constants prep
PREFETCH = 4
# software-pipeline skew (groups) between transpose-MMs and G-MMs on PE
SKEW = 3
# the last XBAR_LAST psum-groups transpose via the DMA xbar (the SDMA
# engines are idle once the x loads finish) instead of PE matmuls + copy
# (measured: a net loss on this workload — keep 0)
XBAR_LAST = 0

F32 = mybir.dt.float32
BF16 = mybir.dt.bfloat16
AF = mybir.ActivationFunctionType
ALU = mybir.AluOpType

_cached_nc = {}


def _emit(ctx: ExitStack, tc, x_d, c_d, st_d, out_d, repeat: int = 1,
          hw_loop: int = 1):
    nc = tc.nc

    const = ctx.enter_context(tc.tile_pool(name="const", bufs=1))
    xpool = ctx.enter_context(tc.tile_pool(name="xin", bufs=PREFETCH + 2))
    xtps = ctx.enter_context(tc.tile_pool(name="xtps", bufs=2, space="PSUM"))
    xtsb = ctx.enter_context(tc.tile_pool(name="xtsb", bufs=SKEW + 3))
    xsqp = ctx.enter_context(tc.tile_pool(name="xsq", bufs=SKEW + 3))
    gps = ctx.enter_context(tc.tile_pool(name="gps", bufs=3, space="PSUM"))
    scr_ps = ctx.enter_context(tc.tile_pool(name="scrps", bufs=1, space="PSUM"))
    endp = ctx.enter_context(tc.tile_pool(name="endp", bufs=1))

    # ---- prefetch first x groups before anything else ------------------
    def x_src(gd):
        src = x_d[gd * DMA_GROUP * P:(gd + 1) * DMA_GROUP * P, :]
        # row (p, t) of group gd = batch gd*1024 + p*8 + t: each partition
        # reads one contiguous 8 KiB run per DMA
        return src.rearrange("(p t) d -> p t d", t=DMA_GROUP)

    prefetched = {}
    for gd in range(PREFETCH):
        xf8 = xpool.tile([P, DMA_GROUP, D], F32, tag="xf")
        if gd == 0:
            # split the first load so the pipeline starts on half a group
            h = DMA_GROUP // 2
            nc.sync.dma_start(xf8[:, :h, :], x_src(gd)[:, :h, :])
            nc.sync.dma_start(xf8[:, h:, :], x_src(gd)[:, h:, :])
        else:
            nc.sync.dma_start(xf8[:], x_src(gd))
        prefetched[gd] = xf8

    # c + st load on the scalar HWDGE ring: independent FIFO, so they are
    # not stuck behind the MiB-scale x prefetches on the sync ring
    c_sb = const.tile([C, D], F32)
    nc.scalar.dma_start(c_sb[:], c_d[:])

    # ---- one-time prep -------------------------------------------------
    ident_bf = const.tile([P, P], BF16)
    make_identity(nc, ident_bf[:])

    # c2 = rowsum(c^2) as a [128, 1] fp32 column
    c_sq = const.tile([C, D], F32)
    c2col = const.tile([C, 1], F32)
    nc.scalar.activation(c_sq[:], c_sb[:], AF.Square, accum_out=c2col[:])

    # (-2c) in bf16, then its transpose cT [d-chunk partitions, k, centers]
    cm2 = const.tile([C, D], BF16)
    nc.vector.tensor_scalar_mul(cm2[:], c_sb[:], -2.0)
    ct_ps = scr_ps.tile([P, 2, C], BF16, tag="scratch")
    for k in range(2):
        nc.tensor.transpose(ct_ps[:, k, :], cm2[:, k * P:(k + 1) * P], ident_bf[:])
    cT = const.tile([P, 2, C], BF16)
    nc.vector.tensor_copy(cT[:], ct_ps[:])

    # c2 as two bf16 K-rows (hi + lo) so a K=2 ones-matmul adds fp32-accurate
    # c2.  Build hi/lo as COLUMNS of a [C, 2] tile (engines can write any
    # free offset but not partition base 1), then one PE transpose makes the
    # [2, C] row pair.  No DMA: a tiny SBUF->SBUF DMA here gets stuck for
    # ~15us behind the MiB-scale x-load packets on the shared SDMA engines.
    c2cols = const.tile([C, 2], BF16)
    nc.vector.tensor_copy(c2cols[:, 0:1], c2col[:])
    c2hi_f = const.tile([C, 1], F32)
    nc.vector.tensor_copy(c2hi_f[:], c2cols[:, 0:1])
    c2lo_f = const.tile([C, 1], F32)
    nc.vector.tensor_tensor(c2lo_f[:], c2col[:], c2hi_f[:], op=ALU.subtract)
    nc.vector.tensor_copy(c2cols[:, 1:2], c2lo_f[:])
    c2t_ps = scr_ps.tile([2, C], F32, tag="scratch")
    nc.tensor.matmul(c2t_ps[:], lhsT=c2cols[:], rhs=ident_bf[:])
    c2rows = const.tile([2, C], BF16)
    nc.vector.tensor_copy(c2rows[:], c2t_ps[:])

    ones2 = const.tile([2, C], BF16)
    nc.vector.memset(ones2[:], 1.0)
    ones_col = const.tile([P, 1], F32)
    nc.vector.memset(ones_col[:], 1.0)
    # all-ones [d, j] rhs for the xsqT matmuls that inject x2 into PSUM
    ones_t = const.tile([P, C], BF16)
    nc.vector.memset(ones_t[:], 1.0)

    # c2rows replicated PSUM_GROUP times for the single N=512 c2 matmul
    c2rows4 = const.tile([2, PSUM_GROUP, C], BF16)
    for i in range(PSUM_GROUP):
        nc.vector.tensor_copy(c2rows4[:, i, :], c2rows[:])

    # semi_target: host pre-reorders it to the x row mapping (batch row
    # i = g*(DMA_GROUP*P) + p*DMA_GROUP + t at st_sb[p, g*DMA_GROUP + t]),
    # so the device load is 512 B contiguous per partition.  The naive
    # gather layout is 32 B/descriptor and crawls behind the x packets.
    st_sb = const.tile([P, NT], F32)
    nc.scalar.dma_start(st_sb[:], st_d.rearrange("(p n) -> p n", p=P))
    # st-derived endgame operands, precomputed while the pipeline is cold
    mneg = const.tile([P, NT], F32)
    nc.vector.tensor_scalar_min(mneg[:], st_sb[:], 0.0)
    epsq = const.tile([P, NT], F32)
    nc.vector.tensor_scalar(epsq[:], st_sb[:], 0.0, EPS, op0=ALU.max, op1=ALU.mult)

    # per-b-tile min columns: column j <-> b-tile j, partition p <-> row
    dw = const.tile([P, NT], F32)

    # ---- main loop -----------------------------------------------------
    pending = []  # (group_idx, xt_sb, xsq) awaiting G-matmuls
    gdone = []    # (group_idx, g_ps) awaiting min-reduce

    def emit_g(g, xt_sb, xsq):
        folded = xsq.shape[2] == 1
        g_ps = gps.tile([P, PSUM_GROUP, C], F32)
        nc.tensor.matmul(
            g_ps[:].rearrange("p t c -> p (t c)"),
            lhsT=ones2[:], rhs=c2rows4[:].rearrange("p t c -> p (t c)"),
            start=True, stop=False,
        )
        for i in range(PSUM_GROUP):
            nc.tensor.matmul(
                g_ps[:, i, :], lhsT=xt_sb[:, i, 0, :], rhs=cT[:, 0, :],
                start=False, stop=False,
            )
            nc.tensor.matmul(
                g_ps[:, i, :], lhsT=xt_sb[:, i, 1, :], rhs=cT[:, 1, :],
                start=False, stop=False,
            )
            nc.tensor.matmul(
                g_ps[:, i, :], lhsT=xsq[:, i, 0, :], rhs=ones_t[:],
                start=False, stop=(folded and i == PSUM_GROUP - 1),
            )
            if not folded:
                nc.tensor.matmul(
                    g_ps[:, i, :], lhsT=xsq[:, i, 1, :], rhs=ones_t[:],
                    start=False, stop=(i == PSUM_GROUP - 1),
                )
        gdone.append((g, g_ps))

    def emit_min(g, g_ps):
        col0 = g * PSUM_GROUP
        nc.vector.tensor_reduce(
            dw[:, col0:col0 + PSUM_GROUP], g_ps[:], axis=mybir.AxisListType.X,
            op=ALU.min,
        )

    with tc.For_i(0, hw_loop, 1) if hw_loop > 1 else nullcontext():
     for _rep in range(repeat):
      for gd in range(NT // DMA_GROUP):
        if gd in prefetched:
            xf8 = prefetched.pop(gd)
        else:
            xf8 = xpool.tile([P, DMA_GROUP, D], F32, tag="xf")
            nc.sync.dma_start(xf8[:], x_src(gd))
        x8 = xpool.tile([P, DMA_GROUP, D], BF16, tag="xb")
        cast_eng = (
            nc.vector.tensor_copy
            if (gd % 10) < CAST_DVE_OF_10
            else nc.scalar.copy
        )
        if gd == 0:
            # split casts to match the split first load: compute starts on
            # the first half-group without waiting for the second
            h = DMA_GROUP // 2
            cast_eng(x8[:, :h, :], xf8[:, :h, :])
            cast_eng(x8[:, h:, :], xf8[:, h:, :])
        else:
            cast_eng(x8[:], xf8[:])

        for gp in range(DMA_GROUP // PSUM_GROUP):
            g = gd * (DMA_GROUP // PSUM_GROUP) + gp
            tiles = [gp * PSUM_GROUP + t for t in range(PSUM_GROUP)]

            use_xbar = g >= 2 * (NT // DMA_GROUP) - XBAR_LAST
            xt_sb = xtsb.tile([P, PSUM_GROUP, 2, P], BF16)
            if use_xbar:
                xt_ps = None
                nc.sync.dma_start_transpose(
                    xt_sb[:].rearrange("p t k b -> p (t k) b"),
                    x8[:, gp * PSUM_GROUP:(gp + 1) * PSUM_GROUP, :]
                    .rearrange("p t d -> p (t d)"),
                )
            else:
                xt_ps = xtps.tile([P, PSUM_GROUP, 2, P], F32)
                for i, t in enumerate(tiles):
                    for k in range(2):
                        nc.tensor.matmul(
                            xt_ps[:, i, k, :],
                            lhsT=x8[:, t, k * P:(k + 1) * P],
                            rhs=ident_bf[:],
                        )
                if (g % 10) < COPY_ACT_OF_10:
                    nc.scalar.copy(xt_sb[:], xt_ps[:])
                else:
                    nc.vector.tensor_copy(xt_sb[:], xt_ps[:])

            xsq = xsqp.tile([P, PSUM_GROUP, 2, P], BF16)
            if (g % 10) < SQ_DVE_OF_10:
                nc.vector.tensor_tensor(xsq[:], xt_sb[:], xt_sb[:], op=ALU.mult)
            elif SQ_FROM_PSUM and xt_ps is not None:
                nc.scalar.activation(xsq[:], xt_ps[:], AF.Square)
            else:
                nc.scalar.activation(xsq[:], xt_sb[:], AF.Square)
            if X2_FOLD:
                xsq_f = xsqp.tile([P, PSUM_GROUP, 1, P], BF16, tag="xsqf")
                eng = nc.gpsimd if X2_FOLD == "gpsimd" else nc.vector
                eng.tensor_tensor(
                    xsq_f[:, :, 0, :], xsq[:, :, 0, :], xsq[:, :, 1, :],
                    op=ALU.add,
                )
                xsq = xsq_f

            pending.append((g, xt_sb, xsq))
            if len(pending) > SKEW:
                emit_g(*pending.pop(0))
            while len(gdone) > 1:
                emit_min(*gdone.pop(0))

      while pending:
        emit_g(*pending.pop(0))
      while gdone:
        emit_min(*gdone.pop(0))

    # ---- endgame -------------------------------------------------------
    dist = endp.tile([P, NT], F32)
    nc.vector.tensor_scalar_max(dist[:], dw[:], 0.0)
    dT = dist

    dp = endp.tile([P, NT], F32)
    nc.vector.tensor_scalar_add(dp[:], dT[:], EPS)
    r = endp.tile([P, NT], F32)
    nc.vector.reciprocal(r[:], dp[:])

    # loss = dT + min(st,0)*(dT - r) + max(st,0)*EPS
    t1 = endp.tile([P, NT], F32)
    nc.vector.tensor_tensor(t1[:], dT[:], r[:], op=ALU.subtract)
    t2 = endp.tile([P, NT], F32)
    nc.vector.tensor_tensor(t2[:], mneg[:], t1[:], op=ALU.mult)
    t3 = endp.tile([P, NT], F32)
    nc.vector.tensor_tensor(t3[:], dT[:], t2[:], op=ALU.add)
    losses = endp.tile([P, NT], F32)
    nc.vector.tensor_tensor(losses[:], t3[:], epsq[:], op=ALU.add)

    lsum = endp.tile([P, 1], F32)
    nc.vector.tensor_reduce(lsum[:], losses[:], axis=mybir.AxisListType.X, op=ALU.add)
    total_ps = scr_ps.tile([1, 1], F32, tag="scratch")
    nc.tensor.matmul(total_ps[:], lhsT=ones_col[:], rhs=lsum[:])
    total_sb = endp.tile([1, 1], F32)
    nc.vector.tensor_copy(total_sb[:], total_ps[:])
    nc.sync.dma_start(out_d[:], total_sb[:])


def build_nc(repeat: int = 1, hw_loop: int = 1, internal_x: bool = False):
    key = (repeat, hw_loop, internal_x)
    if key in _cached_nc:
        return _cached_nc[key]
    nc = bacc.Bacc(
        "TRN2",
        target_bir_lowering=False,
        debug=False,
        enable_asserts=False,
        num_devices=N_CORES,
    )
    if internal_x:
        x_d = nc.dram_tensor("x", [B_SH, D], F32).ap()
    else:
        x_d = nc.dram_tensor("x", [B_SH, D], F32, kind="ExternalInput").ap()
    c_d = nc.dram_tensor("c", [C, D], F32, kind="ExternalInput").ap()
    st_d = nc.dram_tensor("st", [B_SH], F32, kind="ExternalInput").ap()
    out_d = nc.dram_tensor("out", [1, 1], F32, kind="ExternalOutput").ap()

    with tile.TileContext(nc) as tc:
        with ExitStack() as ctx:
            _emit(ctx, tc, x_d, c_d, st_d, out_d, repeat=repeat, hw_loop=hw_loop)
    nc.compile()
    _cached_nc[key] = nc
    return nc


def make_in_maps(x, c, stf):
    def st_reorder(s):
        # st_sb[p, g*DMA_GROUP + t] = s[g*(DMA_GROUP*P) + p*DMA_GROUP + t]
        return np.ascontiguousarray(
            s.reshape(NT // DMA_GROUP, P, DMA_GROUP)
            .transpose(1, 0, 2)
            .reshape(B_SH)
        )

    return [
        {
            "x": np.ascontiguousarray(x[i * B_SH:(i + 1) * B_SH]),
            "c": c,
            "st": st_reorder(stf[i * B_SH:(i + 1) * B_SH]),
        }
        for i in range(N_CORES)
    ]


def kernel(**inputs) -> np.ndarray:
    x = np.ascontiguousarray(np.asarray(inputs["input"], dtype=np.float32))
    c = np.ascontiguousarray(np.asarray(inputs["c"], dtype=np.float32))
    stf = np.asarray(inputs["semi_target"]).astype(np.float32)

    nc = build_nc()
    res = run_bass_kernel_spmd(nc, make_in_maps(x, c, stf), list(range(N_CORES)))
    total = sum(float(r["out"][0, 0]) for r in res.results)
    return np.asarray(np.float32(total / B))


# revision 45
# speedup vs baseline: 1.0377x; 1.0064x over previous
"""DMSAD loss kernel for Trainium2 (8 NeuronCores, data-parallel over batch).

Computes mean over B rows of:
    dist_i = max(min_j ||x_i - c_j||^2, 0)
    loss_i = dist_i                 if st_i == 0
             dist_i + EPS           if st_i == 1
             1 / (dist_i + EPS)     if st_i == -1

Strategy per core (B_SH = 16384 rows, D = 256, C = 128):
  - HWDGE DMA of x fp32 (1 MiB per 8-tile group, 4 groups prefetched,
    first group split for a fast start) is the roofline driver
    (~47-51us/core).  c and semi_target load on the scalar HWDGE ring so
    they are not queued behind the x prefetches; semi_target is
    pre-reordered on the host so its load is 512 B/partition contiguous
    (the gather layout was 32 B/descriptor and crawled behind x packets).
  - DVE casts fp32 -> bf16 (2x_2P mode).
  - PE transposes 128x128 bf16 chunks via matmul-against-identity (keeps
    FWL + warm HAM clock; ~56 ns/pair warm); ACT copy-casts PSUM -> SBUF
    bf16.
  - Squares of the transposed tiles (bf16): DVE tensor_tensor from SBUF
    for 6/10 groups, ACT Square straight from PSUM (parallel with the
    copy) for the rest.
  - Augmented matmul accumulates the FULL distance in PSUM:
        d2[i,j] = sum_d xsqT[d,i]*1  - 2 x.c^T  + c2_j
    via per tile: xT0.cT0, xT1.cT1, xsqT0.ones, xsqT1.ones, plus one
    K=2 N=512 c2 matmul per 4-tile PSUM group (bf16 hi/lo rows keep c2
    fp32-accurate; built via a PE transpose of hi/lo columns -- no DMA).
    No per-tile accumulator reads anywhere.  G-matmuls trail the
    transposes by SKEW groups so the PE stream never stalls on the
    copy/square latency.
  - DVE min-reduces each PSUM group over centers -> dist columns.
  - Endgame on [128, NT]: relu, select by semi_target (st-derived masks
    precomputed at startup), row sums, one ones-matmul partition
    reduction to a scalar.
Host sums the 8 per-core partial sums and divides by global B.
"""

from contextlib import ExitStack, nullcontext

import numpy as np

import concourse.bass as bass
import concourse.tile as tile
from concourse import bacc, mybir
from concourse.bass_utils import run_bass_kernel_spmd
from concourse.masks import make_identity

N_CORES = 8
B = 131072
D = 256
C = 128
P = 128
B_SH = B // N_CORES          # 16384 rows per core
NT = B_SH // P               # 128 b-tiles of 128 rows
PSUM_GROUP = 4               # b-tiles per PSUM batch (one G bank)
DMA_GROUP = 8                # b-tiles per input DMA (1 MiB fp32 reads)
NG = NT // PSUM_GROUP        # 32 psum groups
ETA = 1.0
EPS = 1e-6

# engine balancing knobs -------------------------------------------------
# PSUM->SBUF copy-cast of transposed x: out of 10 groups, this many on ACT
# (rest on DVE).
COPY_ACT_OF_10 = 10
# xsq squaring: out of 10 groups, this many on DVE (rest on ACT Square).
SQ_DVE_OF_10 = 6
# fp32->bf16 input cast: out of 10 DMA-groups, this many on DVE (rest ACT)
CAST_DVE_OF_10 = 10
# fold xsq chunk pairs (xsq0+xsq1) so one N=128 matmul injects x2: engine
# "gpsimd" (idle engine), "dve", or "" to keep two x2-matmuls per tile
X2_FOLD = ""
# ACT-side squares read the transposed tiles from PSUM (runs parallel to
# the PSUM->SBUF copy instead of after it)
SQ_FROM_PSUM = True
# number of x DMA groups issued before the constants prep
PREFETCH = 4
# software-pipeline skew (groups) between transpose-MMs and G-MMs on PE
SKEW = 3
# the last XBAR_LAST psum-groups transpose via the DMA xbar (the SDMA
# engines are idle once the x loads finish) instead of PE matmuls + copy
# (measured: a net loss on this workload — keep 0)
XBAR_LAST = 0

F32 = mybir.dt.float32
BF16 = mybir.dt.bfloat16
AF = mybir.ActivationFunctionType
ALU = mybir.AluOpType

_cached_nc = {}


def _emit(ctx: ExitStack, tc, x_d, c_d, st_d, out_d, repeat: int = 1,
          hw_loop: int = 1):
    nc = tc.nc

    const = ctx.enter_context(tc.tile_pool(name="const", bufs=1))
    xpool = ctx.enter_context(tc.tile_pool(name="xin", bufs=PREFETCH + 2))
    xtps = ctx.enter_context(tc.tile_pool(name="xtps", bufs=2, space="PSUM"))
    xtsb = ctx.enter_context(tc.tile_pool(name="xtsb", bufs=SKEW + 3))
    xsqp = ctx.enter_context(tc.tile_pool(name="xsq", bufs=SKEW + 3))
    gps = ctx.enter_context(tc.tile_pool(name="gps", bufs=3, space="PSUM"))
    scr_ps = ctx.enter_context(tc.tile_pool(name="scrps", bufs=1, space="PSUM"))
    endp = ctx.enter_context(tc.tile_pool(name="endp", bufs=1))

    # ---- prefetch first x groups before anything else ------------------
    def x_src(gd):
        src = x_d[gd * DMA_GROUP * P:(gd + 1) * DMA_GROUP * P, :]
        # row (p, t) of group gd = batch gd*1024 + p*8 + t: each partition
        # reads one contiguous 8 KiB run per DMA
        return src.rearrange("(p t) d -> p t d", t=DMA_GROUP)

    prefetched = {}
    for gd in range(PREFETCH):
        xf8 = xpool.tile([P, DMA_GROUP, D], F32, tag="xf")
        if gd == 0:
            # split the first load so the pipeline starts on half a group
            h = DMA_GROUP // 2
            nc.sync.dma_start(xf8[:, :h, :], x_src(gd)[:, :h, :])
            nc.sync.dma_start(xf8[:, h:, :], x_src(gd)[:, h:, :])
        else:
            nc.sync.dma_start(xf8[:], x_src(gd))
        prefetched[gd] = xf8

    # c + st load on the scalar HWDGE ring: independent FIFO, so they are
    # not stuck behind the MiB-scale x prefetches on the sync ring
    c_sb = const.tile([C, D], F32)
    nc.scalar.dma_start(c_sb[:], c_d[:])

    # ---- one-time prep -------------------------------------------------
    ident_bf = const.tile([P, P], BF16)
    make_identity(nc, ident_bf[:])

    # c2 = rowsum(c^2) as a [128, 1] fp32 column
    c_sq = const.tile([C, D], F32)
    c2col = const.tile([C, 1], F32)
    nc.scalar.activation(c_sq[:], c_sb[:], AF.Square, accum_out=c2col[:])

    # (-2c) in bf16, then its transpose cT [d-chunk partitions, k, centers]
    cm2 = const.tile([C, D], BF16)
    nc.vector.tensor_scalar_mul(cm2[:], c_sb[:], -2.0)
    ct_ps = scr_ps.tile([P, 2, C], BF16, tag="scratch")
    for k in range(2):
        nc.tensor.transpose(ct_ps[:, k, :], cm2[:, k * P:(k + 1) * P], ident_bf[:])
    cT = const.tile([P, 2, C], BF16)
    nc.vector.tensor_copy(cT[:], ct_ps[:])

    # c2 as two bf16 K-rows (hi + lo) so a K=2 ones-matmul adds fp32-accurate
    # c2.  Build hi/lo as COLUMNS of a [C, 2] tile (engines can write any
    # free offset but not partition base 1), then one PE transpose makes the
    # [2, C] row pair.  No DMA: a tiny SBUF->SBUF DMA here gets stuck for
    # ~15us behind the MiB-scale x-load packets on the shared SDMA engines.
    c2cols = const.tile([C, 2], BF16)
    nc.vector.tensor_copy(c2cols[:, 0:1], c2col[:])
    c2hi_f = const.tile([C, 1], F32)
    nc.vector.tensor_copy(c2hi_f[:], c2cols[:, 0:1])
    c2lo_f = const.tile([C, 1], F32)
    nc.vector.tensor_tensor(c2lo_f[:], c2col[:], c2hi_f[:], op=ALU.subtract)
    nc.vector.tensor_copy(c2cols[:, 1:2], c2lo_f[:])
    c2t_ps = scr_ps.tile([2, C], F32, tag="scratch")
    nc.tensor.matmul(c2t_ps[:], lhsT=c2cols[:], rhs=ident_bf[:])
    c2rows = const.tile([2, C], BF16)
    nc.vector.tensor_copy(c2rows[:], c2t_ps[:])

    ones2 = const.tile([2, C], BF16)
    nc.vector.memset(ones2[:], 1.0)
    ones_col = const.tile([P, 1], F32)
    nc.vector.memset(ones_col[:], 1.0)
    # all-ones [d, j] rhs for the xsqT matmuls that inject x2 into PSUM
    ones_t = const.tile([P, C], BF16)
    nc.vector.memset(ones_t[:], 1.0)

    # c2rows replicated PSUM_GROUP times for the single N=512 c2 matmul
    c2rows4 = const.tile([2, PSUM_GROUP, C], BF16)
    for i in range(PSUM_GROUP):
        nc.vector.tensor_copy(c2rows4[:, i, :], c2rows[:])

    # semi_target: host pre-reorders it to the x row mapping (batch row
    # i = g*(DMA_GROUP*P) + p*DMA_GROUP + t at st_sb[p, g*DMA_GROUP + t]),
    # so the device load is 512 B contiguous per partition.  The naive
    # gather layout is 32 B/descriptor and crawls behind the x packets.
    st_sb = const.tile([P, NT], F32)
    nc.scalar.dma_start(st_sb[:], st_d.rearrange("(p n) -> p n", p=P))
    # st-derived endgame operands, precomputed while the pipeline is cold
    mneg = const.tile([P, NT], F32)
    nc.vector.tensor_scalar_min(mneg[:], st_sb[:], 0.0)
    epsq = const.tile([P, NT], F32)
    nc.vector.tensor_scalar(epsq[:], st_sb[:], 0.0, EPS, op0=ALU.max, op1=ALU.mult)

    # per-b-tile min columns: column j <-> b-tile j, partition p <-> row
    dw = const.tile([P, NT], F32)

    # ---- main loop -----------------------------------------------------
    pending = []  # (group_idx, xt_sb, xsq) awaiting G-matmuls
    gdone = []    # (group_idx, g_ps) awaiting min-reduce

    # endgame for a column range of dw (dist -> per-row loss), emitted in
    # halves so most of it overlaps the main loop instead of the tail
    def emit_endgame(lo, hi, part):
        w = hi - lo
        dist = endp.tile([P, w], F32, tag=f"dist{part}")
        nc.vector.tensor_scalar_max(dist[:], dw[:, lo:hi], 0.0)
        dp = endp.tile([P, w], F32, tag=f"dp{part}")
        nc.vector.tensor_scalar_add(dp[:], dist[:], EPS)
        r = endp.tile([P, w], F32, tag=f"r{part}")
        nc.vector.reciprocal(r[:], dp[:])
        t1 = endp.tile([P, w], F32, tag=f"t1{part}")
        nc.vector.tensor_tensor(t1[:], dist[:], r[:], op=ALU.subtract)
        t2 = endp.tile([P, w], F32, tag=f"t2{part}")
        nc.vector.tensor_tensor(t2[:], mneg[:, lo:hi], t1[:], op=ALU.mult)
        t3 = endp.tile([P, w], F32, tag=f"t3{part}")
        nc.vector.tensor_tensor(t3[:], dist[:], t2[:], op=ALU.add)
        losses = endp.tile([P, w], F32, tag=f"l{part}")
        nc.vector.tensor_tensor(losses[:], t3[:], epsq[:, lo:hi], op=ALU.add)
        lsum = endp.tile([P, 1], F32, tag=f"ls{part}")
        nc.vector.tensor_reduce(
            lsum[:], losses[:], axis=mybir.AxisListType.X, op=ALU.add
        )
        return lsum

    def emit_g(g, xt_sb, xsq):
        folded = xsq.shape[2] == 1
        g_ps = gps.tile([P, PSUM_GROUP, C], F32)
        nc.tensor.matmul(
            g_ps[:].rearrange("p t c -> p (t c)"),
            lhsT=ones2[:], rhs=c2rows4[:].rearrange("p t c -> p (t c)"),
            start=True, stop=False,
        )
        for i in range(PSUM_GROUP):
            nc.tensor.matmul(
                g_ps[:, i, :], lhsT=xt_sb[:, i, 0, :], rhs=cT[:, 0, :],
                start=False, stop=False,
            )
            nc.tensor.matmul(
                g_ps[:, i, :], lhsT=xt_sb[:, i, 1, :], rhs=cT[:, 1, :],
                start=False, stop=False,
            )
            nc.tensor.matmul(
                g_ps[:, i, :], lhsT=xsq[:, i, 0, :], rhs=ones_t[:],
                start=False, stop=(folded and i == PSUM_GROUP - 1),
            )
            if not folded:
                nc.tensor.matmul(
                    g_ps[:, i, :], lhsT=xsq[:, i, 1, :], rhs=ones_t[:],
                    start=False, stop=(i == PSUM_GROUP - 1),
                )
        gdone.append((g, g_ps))

    def emit_min(g, g_ps):
        col0 = g * PSUM_GROUP
        nc.vector.tensor_reduce(
            dw[:, col0:col0 + PSUM_GROUP], g_ps[:], axis=mybir.AxisListType.X,
            op=ALU.min,
        )

    with tc.For_i(0, hw_loop, 1) if hw_loop > 1 else nullcontext():
     for _rep in range(repeat):
      for gd in range(NT // DMA_GROUP):
        if gd in prefetched:
            xf8 = prefetched.pop(gd)
        else:
            xf8 = xpool.tile([P, DMA_GROUP, D], F32, tag="xf")
            nc.sync.dma_start(xf8[:], x_src(gd))
        x8 = xpool.tile([P, DMA_GROUP, D], BF16, tag="xb")
        cast_eng = (
            nc.vector.tensor_copy
            if (gd % 10) < CAST_DVE_OF_10
            else nc.scalar.copy
        )
        if gd == 0:
            # split casts to match the split first load: compute starts on
            # the first half-group without waiting for the second
            h = DMA_GROUP // 2
            cast_eng(x8[:, :h, :], xf8[:, :h, :])
            cast_eng(x8[:, h:, :], xf8[:, h:, :])
        else:
            cast_eng(x8[:], xf8[:])

        for gp in range(DMA_GROUP // PSUM_GROUP):
            g = gd * (DMA_GROUP // PSUM_GROUP) + gp
            tiles = [gp * PSUM_GROUP + t for t in range(PSUM_GROUP)]

            use_xbar = g >= 2 * (NT // DMA_GROUP) - XBAR_LAST
            xt_sb = xtsb.tile([P, PSUM_GROUP, 2, P], BF16)
            if use_xbar:
                xt_ps = None
                nc.sync.dma_start_transpose(
                    xt_sb[:].rearrange("p t k b -> p (t k) b"),
                    x8[:, gp * PSUM_GROUP:(gp + 1) * PSUM_GROUP, :]
                    .rearrange("p t d -> p (t d)"),
                )
            else:
                xt_ps = xtps.tile([P, PSUM_GROUP, 2, P], F32)
                for i, t in enumerate(tiles):
                    for k in range(2):
                        nc.tensor.matmul(
                            xt_ps[:, i, k, :],
                            lhsT=x8[:, t, k * P:(k + 1) * P],
                            rhs=ident_bf[:],
                        )
                if (g % 10) < COPY_ACT_OF_10:
                    nc.scalar.copy(xt_sb[:], xt_ps[:])
                else:
                    nc.vector.tensor_copy(xt_sb[:], xt_ps[:])

            xsq = xsqp.tile([P, PSUM_GROUP, 2, P], BF16)
            if (g % 10) < SQ_DVE_OF_10:
                nc.vector.tensor_tensor(xsq[:], xt_sb[:], xt_sb[:], op=ALU.mult)
            elif SQ_FROM_PSUM and xt_ps is not None:
                nc.scalar.activation(xsq[:], xt_ps[:], AF.Square)
            else:
                nc.scalar.activation(xsq[:], xt_sb[:], AF.Square)
            if X2_FOLD:
                xsq_f = xsqp.tile([P, PSUM_GROUP, 1, P], BF16, tag="xsqf")
                eng = nc.gpsimd if X2_FOLD == "gpsimd" else nc.vector
                eng.tensor_tensor(
                    xsq_f[:, :, 0, :], xsq[:, :, 0, :], xsq[:, :, 1, :],
                    op=ALU.add,
                )
                xsq = xsq_f

            pending.append((g, xt_sb, xsq))
            # during the DMA-paced ramp groups arrive slowly (copy/sq are
            # long done), so drain the skew window early to start G sooner
            skew_now = 1 if g < 4 else SKEW
            while len(pending) > skew_now:
                emit_g(*pending.pop(0))
            while len(gdone) > 1:
                emit_min(*gdone.pop(0))
            if g == NG // 2 + SKEW + 1:
                # first half of dw is min-reduced; overlap its endgame
                lsum1 = emit_endgame(0, NT // 2, 0)

      while pending:
        emit_g(*pending.pop(0))
      while gdone:
        emit_min(*gdone.pop(0))

    # ---- endgame (second half; first half overlapped the main loop) ----
    lsum2 = emit_endgame(NT // 2, NT, 1)
    lsum = endp.tile([P, 1], F32, tag="lsum")
    nc.vector.tensor_tensor(lsum[:], lsum1[:], lsum2[:], op=ALU.add)
    total_ps = scr_ps.tile([1, 1], F32, tag="scratch")
    nc.tensor.matmul(total_ps[:], lhsT=ones_col[:], rhs=lsum[:])
    total_sb = endp.tile([1, 1], F32)
    nc.vector.tensor_copy(total_sb[:], total_ps[:])
    nc.sync.dma_start(out_d[:], total_sb[:])


def build_nc(repeat: int = 1, hw_loop: int = 1, internal_x: bool = False):
    key = (repeat, hw_loop, internal_x)
    if key in _cached_nc:
        return _cached_nc[key]
    nc = bacc.Bacc(
        "TRN2",
        target_bir_lowering=False,
        debug=False,
        enable_asserts=False,
        num_devices=N_CORES,
    )
    if internal_x:
        x_d = nc.dram_tensor("x", [B_SH, D], F32).ap()
    else:
        x_d = nc.dram_tensor("x", [B_SH, D], F32, kind="ExternalInput").ap()
    c_d = nc.dram_tensor("c", [C, D], F32, kind="ExternalInput").ap()
    st_d = nc.dram_tensor("st", [B_SH], F32, kind="ExternalInput").ap()
    out_d = nc.dram_tensor("out", [1, 1], F32, kind="ExternalOutput").ap()

    with tile.TileContext(nc) as tc:
        with ExitStack() as ctx:
            _emit(ctx, tc, x_d, c_d, st_d, out_d, repeat=repeat, hw_loop=hw_loop)
    nc.compile()
    _cached_nc[key] = nc
    return nc


def make_in_maps(x, c, stf):
    def st_reorder(s):
        # st_sb[p, g*DMA_GROUP + t] = s[g*(DMA_GROUP*P) + p*DMA_GROUP + t]
        return np.ascontiguousarray(
            s.reshape(NT // DMA_GROUP, P, DMA_GROUP)
            .transpose(1, 0, 2)
            .reshape(B_SH)
        )

    return [
        {
            "x": np.ascontiguousarray(x[i * B_SH:(i + 1) * B_SH]),
            "c": c,
            "st": st_reorder(stf[i * B_SH:(i + 1) * B_SH]),
        }
        for i in range(N_CORES)
    ]


def kernel(**inputs) -> np.ndarray:
    x = np.ascontiguousarray(np.asarray(inputs["input"], dtype=np.float32))
    c = np.ascontiguousarray(np.asarray(inputs["c"], dtype=np.float32))
    stf = np.asarray(inputs["semi_target"]).astype(np.float32)

    nc = build_nc()
    res = run_bass_kernel_spmd(nc, make_in_maps(x, c, stf), list(range(N_CORES)))
    total = sum(float(r["out"][0, 0]) for r in res.results)
    return np.asarray(np.float32(total / B))
